# revision 12
# baseline (speedup 1.0000x reference)
"""Block-diagonal linear (grouped GEMM) on 8 TRN2 NeuronCores.

out[b, g*512+n] = sum_k x[b, g*512+k] * blocks[g, k, n]

Group-parallel: core g computes block g's GEMM in bf16 with fp32 PSUM
accumulation (end-to-end max rel err ~4.2e-3 vs the fp32 reference).
The host hands each core xT = x[:, g*512:(g+1)*512].T ([512, 8192],
feature-major, bf16) and w bf16, and receives outT ([512, 8192] bf16);
transposes/dtype conversion happen on the host so every DMA moves long
contiguous runs per partition and the device needs no PE transposes.

The PE stream itself is at the bf16 roofline (256 matmuls of 512
columns at 216ns warm spacing = 55.3us at 2.4GHz); the schedule exists
to bury everything else (measured exec window = first non-bookkeeping
instruction .. last instruction, which includes a fixed ~8.6us NEFF
semaphore-teardown after the last DMA):
 - the head is input-supply bound (~200GB/s effective early delivery
   on the two HWDGE rings, transfers starting ~1.5us after the first
   issue): W k-tiles interleave with x k-tiles in j-consumption order
   across the rings — pairs (w0|x0), (x1|w1), (w2|x2), (x3|w3) — so
   the j-outer first chunk starts as soon as the first pair lands.
 - warmup dummy matmuls sized to the issue->data window keep the PE
   busy so the HAM clock-gate lifts right as real work starts.
 - all PSUM->SBUF casts on DVE; outp pool bufs=3 so a tail cast never
   waits on a previous chunk's output DMA buffer.
 - mid-kernel outputs ride the SWDGE ring (HW rings carry input);
   the taper ends in a 64-token chunk with all 4 nt-groups packed in
   one PSUM bank: one cast + one fused DMA after the last matmul.

(X_FP8=True sends x as fp8_e3m4 — numerically fine at rel err 1.4e-2
and the PE computes bf16(stationary) x fp8(moving) exactly at full
rate in isolation, but whole-kernel runs intermittently downclock the
PE to 2.0GHz, so bf16 is the safer default.)
"""
import numpy as np
import ml_dtypes

import concourse.bacc as bacc
import concourse.tile as tile
from concourse import mybir
from concourse.bass_utils import run_bass_kernel_spmd

X_FP8 = False  # x dtype: fp8_e3m4 (halves input DMA) vs bf16

TOKENS = 8192
G = 8
M = 512
N = 512
P = 128
KT = M // P
NT = N // P
SUB = 512
F32 = mybir.dt.float32
BF16 = mybir.dt.bfloat16
XDT = mybir.dt.float8e3 if X_FP8 else mybir.dt.bfloat16
NPBF16 = ml_dtypes.bfloat16
NPX = ml_dtypes.float8_e3m4 if X_FP8 else ml_dtypes.bfloat16

CHUNKS = [512, 512, 1024, 2048, 2048, 1024, 512, 256, 192, 64]
assert sum(CHUNKS) == TOKENS
CMAX = max(CHUNKS)
CLAST = CHUNKS[-1]
N_WARM = 20

_CACHE: dict = {}


def _body(tc, nc, xT, w, outT):
    with (
        tc.tile_pool(name="wp", bufs=1) as wp,
        tc.tile_pool(name="xin", bufs=12) as xin,
        tc.tile_pool(name="outp", bufs=3) as outp,
        tc.tile_pool(name="pso", bufs=8, space="PSUM") as pso,
    ):
        w_r = wp.tile([P, KT, N], BF16, tag="wr")
        w_v = w.rearrange("(j p) n -> j p n", p=P)
        outT_v = outT.rearrange("(nt p) t -> p nt t", p=P)

        # Head DMAs interleaved in j-consumption order: ring pairs
        # (w0|x0), (x1|w1), (w2|x2), (x3|w3) land together, so each
        # j k-tile unblocks right as the matmul stream reaches it.
        c0 = CHUNKS[0]
        xs0 = [xin.tile([P, CMAX], XDT, tag="x", name=f"x0_{j}") for j in range(KT)]
        nc.sync.dma_start(w_r[:, 0, :], w_v[0])
        nc.scalar.dma_start(xs0[0][:, :c0], xT[0 * P:1 * P, 0:c0])
        nc.scalar.dma_start(w_r[:, 1, :], w_v[1])
        nc.sync.dma_start(xs0[1][:, :c0], xT[1 * P:2 * P, 0:c0])
        nc.sync.dma_start(w_r[:, 2, :], w_v[2])
        nc.scalar.dma_start(xs0[2][:, :c0], xT[2 * P:3 * P, 0:c0])
        nc.scalar.dma_start(w_r[:, 3, :], w_v[3])
        nc.sync.dma_start(xs0[3][:, :c0], xT[3 * P:4 * P, 0:c0])

        # HAM warm-up: dependency-free dummy matmuls over zeroed SBUF
        # into a scratch PSUM bank while the first DMAs land.
        warm_x = xin.tile([P, CMAX], XDT, tag="x")
        warm_ps = pso.tile([P, SUB], F32, tag="pso")
        nc.vector.memset(warm_x[:, :2 * P], 0)
        for _ in range(N_WARM):
            nc.tensor.matmul(
                warm_ps[:, :P], warm_x[:, :P], warm_x[:, P:2 * P],
                start=True, stop=True,
            )

        m0 = 0
        for ci, c in enumerate(CHUNKS):
            if ci == 0:
                xs = xs0
            else:
                xs = []
                for j in range(KT):
                    x_t = xin.tile([P, CMAX], XDT, tag="x")
                    eng = nc.sync if j % 2 == 0 else nc.scalar
                    eng.dma_start(x_t[:, :c], xT[j * P:(j + 1) * P, m0:m0 + c])
                    xs.append(x_t)

            if ci == len(CHUNKS) - 1:
                # final 64-token chunk: all four nt-groups in ONE PSUM
                # bank; one cast + one fused DMA after the last matmul
                ps_o = pso.tile([P, SUB], F32, tag="pso")
                for nt in range(NT):
                    for j in range(KT):
                        nc.tensor.matmul(
                            ps_o[:, nt * c:(nt + 1) * c],
                            w_r[:, j, nt * P:(nt + 1) * P],
                            xs[j][:, :c],
                            start=(j == 0),
                            stop=(j == KT - 1),
                        )
                otl = outp.tile([P, NT * CLAST], BF16, tag="ol", name="otl")
                nc.vector.tensor_copy(otl, ps_o[:, :NT * c])
                otl_v = otl.rearrange("p (nt c) -> p nt c", nt=NT)
                nc.sync.dma_start(outT_v[:, :, m0:m0 + c], otl_v)
                m0 += c
                continue

            ots = [outp.tile([P, CMAX], BF16, tag=f"o{nt}", name=f"ot{nt}") for nt in range(NT)]
            if ci == 0:
                # j-outer: the first 4 matmuls need only w0+x0
                pss0 = [
                    pso.tile([P, SUB], F32, tag="pso", name=f"ps0_{nt}")
                    for nt in range(NT)
                ]
                for j in range(KT):
                    for nt in range(NT):
                        nc.tensor.matmul(
                            pss0[nt][:, :c],
                            w_r[:, j, nt * P:(nt + 1) * P],
                            xs[j][:, :c],
                            start=(j == 0),
                            stop=(j == KT - 1),
                        )
                for nt in range(NT):
                    nc.vector.tensor_copy(ots[nt][:, :c], pss0[nt][:, :c])
            else:
                for s0 in range(0, c, SUB):
                    sw = min(SUB, c - s0)
                    for nt in range(NT):
                        ps_o = pso.tile([P, SUB], F32, tag="pso")
                        for j in range(KT):
                            nc.tensor.matmul(
                                ps_o[:, :sw],
                                w_r[:, j, nt * P:(nt + 1) * P],
                                xs[j][:, s0:s0 + sw],
                                start=(j == 0),
                                stop=(j == KT - 1),
                            )
                        nc.vector.tensor_copy(ots[nt][:, s0:s0 + sw], ps_o[:, :sw])
            # flush the chunk: one DMA per n-tile; SWDGE ring mid-kernel,
            # HWDGE rings for the tail (input traffic done by then)
            for nt in range(NT):
                if ci >= len(CHUNKS) - 3:
                    eng = nc.sync if nt % 2 == 0 else nc.scalar
                else:
                    eng = nc.gpsimd
                eng.dma_start(outT[nt * P:(nt + 1) * P, m0:m0 + c], ots[nt][:, :c])
            m0 += c


def _build():
    nc = bacc.Bacc("TRN2", target_bir_lowering=False, debug=False, num_devices=G)
    xT = nc.dram_tensor("xT", [M, TOKENS], XDT, kind="ExternalInput").ap()
    w = nc.dram_tensor("w", [M, N], BF16, kind="ExternalInput").ap()
    outT = nc.dram_tensor("outT", [N, TOKENS], BF16, kind="ExternalOutput").ap()
    with tile.TileContext(nc) as tc:
        _body(tc, nc, xT, w, outT)
    nc.compile()
    return nc


def _run(in_maps, **kwargs):
    if "nc" not in _CACHE:
        _CACHE["nc"] = _build()
    return run_bass_kernel_spmd(_CACHE["nc"], in_maps, list(range(G)), **kwargs)


def _in_maps(x, blocks):
    return [
        {
            "xT": np.ascontiguousarray(x[:, g * M:(g + 1) * M].T).astype(NPX),
            "w": np.ascontiguousarray(blocks[g]).astype(NPBF16),
        }
        for g in range(G)
    ]


def kernel(x, blocks):
    x = np.asarray(x)
    blocks = np.asarray(blocks)
    res = _run(_in_maps(x, blocks))
    return np.concatenate(
        [res.results[g]["outT"].T.astype(np.float32) for g in range(G)], axis=1
    )
